# revision 1
# baseline (speedup 1.0000x reference)
"""AutoRec forward pass on 8 Trainium2 NeuronCores (SPMD, no collectives).

Computation (reference):
    z = segment_sum(r[:,None] * V[u], i, num_segments=m)   # (m, D) sparse spmm
    h = sigmoid(z + mu)
    out = sum(h[j] * W[v], -1) + b[v]                      # (n_out,)

Strategy:
  - Users range-sharded over the 8 cores (m/8 each); edges bucketed host-side
    by (core, 128-user tile) so each core owns its z/h rows -> no collectives.
  - Phase 1, per user tile: V[u] rows fetched 128/instruction via indirect
    DMA (one row per partition, the HW-supported form); the segment scatter is
    zT += Vg^T @ S on the PE, where S[e,s] = (i_e-tile_base==s)*r_e is built
    on DVE from an iota tile in one fused tensor_scalar op. mu enters the
    same PSUM group as a rank-1 matmul; sigmoid (ACT) writes a column slice
    of a transposed h buffer hT [128 d, users] that stays RESIDENT IN SBUF.
  - Phase 2: pairs sharded by user, grouped per 128-user window, sorted by v
    inside each window. Per 128-pair block: gather W[v] rows (the only DMA
    gather left), PE-transpose them, QT[k,s] = WgT^T @ hT_window on PE, then
    one tensor_tensor_reduce with the one-hot E[k,s]=(jrel_k==s) picks
    QT[k, jrel_k] per pair and folds in the b[v] bias as the reduce init.
  - Host does index bucketing/permutation + the tiny b[v] lookup table.
"""

import math
import sys

sys.path.insert(0, "/opt/trn_rl_repo")

import numpy as np

D = 128
M_CORES = 8

_PROGRAM_CACHE: dict = {}


def _build_program(NI, T1, B, PB):
    """Build + compile the SPMD Bass program (identical on all cores).

    NI: item count; T1: user tiles/core; B: edge blocks per user tile;
    PB: pair blocks per user window.
    """
    import concourse.bacc as bacc
    import concourse.bass as bass
    import concourse.mybir as mybir
    from concourse.tile import TileContext

    f32 = mybir.dt.float32
    bf16 = mybir.dt.bfloat16
    i32 = mybir.dt.int32
    ALU = mybir.AluOpType
    ACT = mybir.ActivationFunctionType

    nc = bacc.Bacc("TRN2", target_bir_lowering=False, debug=False)

    V_d = nc.dram_tensor("V", [NI, D], f32, kind="ExternalInput")
    W_d = nc.dram_tensor("W", [NI, D], f32, kind="ExternalInput")
    mu_d = nc.dram_tensor("mu", [1, D], f32, kind="ExternalInput")
    eidx_d = nc.dram_tensor("eidx", [T1, 128, B], i32, kind="ExternalInput")
    erel_d = nc.dram_tensor("erel", [T1, 128, B], f32, kind="ExternalInput")
    erat_d = nc.dram_tensor("erat", [T1, 128, B], f32, kind="ExternalInput")
    pv_d = nc.dram_tensor("pv", [T1, 128, PB], i32, kind="ExternalInput")
    PB4 = ((PB + 3) // 4) * 4
    pjrelr_d = nc.dram_tensor("pjrelr", [T1, 1, PB4 * 128], f32, kind="ExternalInput")
    pb_d = nc.dram_tensor("pb", [T1, 128, PB], f32, kind="ExternalInput")
    res_d = nc.dram_tensor("res", [T1, 128, PB], f32, kind="ExternalOutput")

    with TileContext(nc) as tc:
        with tc.tile_pool(name="const", bufs=1) as constp:
            iota_i = constp.tile([128, 128], i32)
            nc.gpsimd.iota(iota_i[:], pattern=[[1, 128]], base=0, channel_multiplier=0)
            iota_f = constp.tile([128, 128], f32)
            nc.vector.tensor_copy(iota_f[:], iota_i[:])
            iotac_i = constp.tile([128, 1], i32)
            nc.gpsimd.iota(iotac_i[:], pattern=[[1, 1]], base=0, channel_multiplier=1)
            iotac_f = constp.tile([128, 1], f32)
            nc.vector.tensor_copy(iotac_f[:], iotac_i[:])
            ones1 = constp.tile([1, 128], f32)
            nc.vector.memset(ones1[:], 1.0)
            mu_t = constp.tile([1, 128], f32)
            nc.sync.dma_start(out=mu_t[:], in_=mu_d[:])
            hres = constp.tile([128, T1 * 128], f32)  # resident h: [user-in-tile, (t,d)]

            # -------- phase 1: hres[:, t] = sigmoid(S^T @ V[u] + mu) per user tile
            with tc.tile_pool(name="p1meta", bufs=3) as mp, \
                 tc.tile_pool(name="p1g", bufs=12) as gp, \
                 tc.tile_pool(name="p1s", bufs=6) as spool, \
                 tc.tile_pool(name="p1z", bufs=2, space="PSUM") as pp:
                for t in range(T1):
                    it = mp.tile([128, B], i32, tag="it")
                    nc.sync.dma_start(out=it[:], in_=eidx_d[t])
                    rel = mp.tile([128, B], f32, tag="rel")
                    nc.sync.dma_start(out=rel[:], in_=erel_d[t])
                    rat = mp.tile([128, B], f32, tag="rat")
                    nc.sync.dma_start(out=rat[:], in_=erat_d[t])
                    zt = pp.tile([128, 128], f32)
                    # z[s, d] = mu[d] (starts the accumulation group; fp32 exact)
                    nc.tensor.matmul(zt[:], lhsT=ones1[:], rhs=mu_t[:],
                                     start=True, stop=False)
                    for bb in range(B):
                        g = gp.tile([128, D], bf16, tag="g")
                        nc.gpsimd.indirect_dma_start(
                            out=g[:], out_offset=None, in_=V_d[:],
                            in_offset=bass.IndirectOffsetOnAxis(
                                ap=it[:, bb:bb + 1], axis=0))
                        S = spool.tile([128, 128], bf16, tag="S")
                        nc.vector.tensor_scalar(
                            out=S[:], in0=iota_f[:],
                            scalar1=rel[:, bb:bb + 1], scalar2=rat[:, bb:bb + 1],
                            op0=ALU.is_equal, op1=ALU.mult)
                        nc.tensor.matmul(zt[:], lhsT=S[:], rhs=g[:],
                                         start=False, stop=(bb == B - 1))
                    nc.scalar.activation(hres[:, t * 128:(t + 1) * 128], zt[:],
                                         ACT.Sigmoid)

            # -------- phase 2: res = b + sum_d (E^T @ h_window)[k,d] * W[v][k,d]
            with tc.tile_pool(name="p2meta", bufs=3) as mp2, \
                 tc.tile_pool(name="p2g", bufs=12) as gp2, \
                 tc.tile_pool(name="p2e", bufs=6) as ep, \
                 tc.tile_pool(name="p2sc", bufs=6) as scp, \
                 tc.tile_pool(name="p2r", bufs=3) as rp, \
                 tc.tile_pool(name="p2jb", bufs=2, space="PSUM") as pjb, \
                 tc.tile_pool(name="p2q", bufs=2, space="PSUM") as pq:
                for t in range(T1):
                    vt = mp2.tile([128, PB], i32, tag="vt")
                    nc.sync.dma_start(out=vt[:], in_=pv_d[t])
                    jrow = mp2.tile([1, PB4 * 128], f32, tag="jrow")
                    nc.sync.dma_start(out=jrow[:], in_=pjrelr_d[t])
                    pbt = mp2.tile([128, PB], f32, tag="pbt")
                    nc.sync.dma_start(out=pbt[:], in_=pb_d[t])
                    rt = rp.tile([128, PB], f32, tag="rt")
                    jb = None
                    for bb in range(PB):
                        wg = gp2.tile([128, D], f32, tag="wg")
                        nc.gpsimd.indirect_dma_start(
                            out=wg[:], out_offset=None, in_=W_d[:],
                            in_offset=bass.IndirectOffsetOnAxis(
                                ap=vt[:, bb:bb + 1], axis=0))
                        if bb % 4 == 0:
                            # JB[s, k] = jrel_k for every s, 4 blocks at a time
                            jb = pjb.tile([128, 512], f32, tag="jb")
                            nc.tensor.matmul(jb[:], lhsT=ones1[:],
                                             rhs=jrow[:, bb * 128:bb * 128 + 512],
                                             start=True, stop=True)
                        # E[s, k] = (jrel_k == s)
                        E = ep.tile([128, 128], f32, tag="E")
                        nc.vector.tensor_scalar(
                            out=E[:], in0=jb[:, (bb % 4) * 128:(bb % 4 + 1) * 128],
                            scalar1=iotac_f[:], scalar2=None,
                            op0=ALU.is_equal)
                        # Hsel[k, d] = h[jrel_k, d] for this window
                        hs = pq.tile([128, 128], f32)
                        nc.tensor.matmul(hs[:], lhsT=E[:],
                                         rhs=hres[:, t * 128:(t + 1) * 128],
                                         start=True, stop=True)
                        sc = scp.tile([128, 128], f32, tag="sc")
                        nc.vector.scalar_tensor_tensor(
                            out=sc[:], in0=hs[:], scalar=1.0, in1=wg[:],
                            op0=ALU.mult, op1=ALU.mult,
                            accum_out=rt[:, bb:bb + 1])
                    nc.vector.tensor_add(rt[:], rt[:], pbt[:])
                    nc.sync.dma_start(out=res_d[t], in_=rt[:])

    nc.compile()
    return nc


def _prep_inputs(u, i, r, m, v, j, V, mu, W, b):
    """Host-side sharding. Returns per-core input maps + unshard info."""
    NU = int(m)
    NI = int(V.shape[0])
    NOUT = int(v.shape[0])
    UC = (NU + M_CORES - 1) // M_CORES       # users per core
    T1 = (UC + 127) // 128                   # 128-user tiles per core

    u32 = np.asarray(u).astype(np.int32)
    i32 = np.asarray(i).astype(np.int32)
    r32 = np.asarray(r, dtype=np.float32)

    c_e = i32 // UC
    tloc = (i32 - c_e * UC) >> 7
    gtile = c_e * T1 + tloc
    irel = (i32 - c_e * UC - (tloc << 7)).astype(np.float32)
    order = np.lexsort((u32, gtile))
    gt_s = gtile[order]
    us = u32[order]
    rs = r32[order]
    irel_s = irel[order]

    NT = M_CORES * T1
    counts = np.bincount(gt_s, minlength=NT)
    B = max(1, int(math.ceil(counts.max() / 128)))
    starts = np.zeros(NT + 1, np.int64)
    np.cumsum(counts, out=starts[1:])

    eidx = np.zeros((M_CORES, T1, 128, B), np.int32)
    erel = np.zeros((M_CORES, T1, 128, B), np.float32)
    erat = np.zeros((M_CORES, T1, 128, B), np.float32)
    for gidx in range(NT):
        s, e = int(starts[gidx]), int(starts[gidx + 1])
        k = e - s
        if k == 0:
            continue
        c, t = divmod(gidx, T1)
        pu = np.zeros(B * 128, np.int32)
        pu[:k] = us[s:e]
        pr = np.zeros(B * 128, np.float32)
        pr[:k] = irel_s[s:e]
        pa = np.zeros(B * 128, np.float32)
        pa[:k] = rs[s:e]
        eidx[c, t] = pu.reshape(B, 128).T
        erel[c, t] = pr.reshape(B, 128).T
        erat[c, t] = pa.reshape(B, 128).T

    # ---- decode pairs: bucket by (core, 128-user window), sort by v inside
    j32 = np.asarray(j).astype(np.int32)
    v32 = np.asarray(v).astype(np.int32)
    bvec = np.asarray(b, dtype=np.float32).reshape(-1)
    cj = j32 // UC
    tj = (j32 - cj * UC) >> 7
    gwin = cj * T1 + tj
    jrel_all = (j32 - cj * UC - (tj << 7)).astype(np.float32)
    order2 = np.lexsort((v32, gwin))
    gw_s = gwin[order2]
    v_s = v32[order2]
    jr_s = jrel_all[order2]
    pb_s = bvec[v_s]

    counts2 = np.bincount(gw_s, minlength=NT)
    PB = max(1, int(math.ceil(counts2.max() / 128)))
    st2 = np.zeros(NT + 1, np.int64)
    np.cumsum(counts2, out=st2[1:])

    PB4 = ((PB + 3) // 4) * 4
    pv = np.zeros((M_CORES, T1, 128, PB), np.int32)
    pjrelr = np.zeros((M_CORES, T1, 1, PB4 * 128), np.float32)
    pb = np.zeros((M_CORES, T1, 128, PB), np.float32)
    for gidx in range(NT):
        s, e = int(st2[gidx]), int(st2[gidx + 1])
        k = e - s
        if k == 0:
            continue
        c, t = divmod(gidx, T1)
        a = np.zeros(PB * 128, np.int32)
        a[:k] = v_s[s:e]
        bbuf = np.zeros(PB * 128, np.float32)
        bbuf[:k] = jr_s[s:e]
        cbuf = np.zeros(PB * 128, np.float32)
        cbuf[:k] = pb_s[s:e]
        pv[c, t] = a.reshape(PB, 128).T
        pjrelr[c, t, 0, :PB * 128] = bbuf
        pb[c, t] = cbuf.reshape(PB, 128).T

    Vf = np.ascontiguousarray(V, dtype=np.float32)
    Wf = np.ascontiguousarray(W, dtype=np.float32)
    muf = np.ascontiguousarray(np.asarray(mu).reshape(1, D), dtype=np.float32)

    in_maps = []
    for c in range(M_CORES):
        in_maps.append({
            "V": Vf, "W": Wf, "mu": muf,
            "eidx": eidx[c], "erel": erel[c], "erat": erat[c],
            "pv": pv[c], "pjrelr": pjrelr[c], "pb": pb[c],
        })
    meta = dict(NI=NI, T1=T1, B=B, PB=PB, NOUT=NOUT,
                counts2=counts2, st2=st2, order2=order2)
    return in_maps, meta


def _unshard(results, meta):
    T1, PB = meta["T1"], meta["PB"]
    counts2 = meta["counts2"]
    order2 = meta["order2"]
    NT = M_CORES * T1
    parts = []
    for gidx in range(NT):
        c, t = divmod(gidx, T1)
        k = int(counts2[gidx])
        if k == 0:
            continue
        flat = results[c]["res"][t].T.reshape(-1)  # (p, bb) -> bb*128+p
        parts.append(flat[:k])
    out = np.empty(meta["NOUT"], np.float32)
    out[order2] = np.concatenate(parts) if parts else np.empty(0, np.float32)
    return out


def run(u, i, r, m, v, j, V, mu, W, b, trace=False, trace_kwargs=None):
    """Full pipeline; returns (out, BassKernelResults)."""
    from concourse import bass_utils

    in_maps, meta = _prep_inputs(u, i, r, m, v, j, V, mu, W, b)
    key = (meta["NI"], meta["T1"], meta["B"], meta["PB"])
    nc = _PROGRAM_CACHE.get(key)
    if nc is None:
        nc = _build_program(*key)
        _PROGRAM_CACHE[key] = nc
    res = bass_utils.run_bass_kernel_spmd(
        nc, in_maps, list(range(M_CORES)), trace=trace, **(trace_kwargs or {}))
    return _unshard(res.results, meta), res


def kernel(u, i, r, m, v, j, V, mu, W, b):
    out, _ = run(u, i, r, m, v, j, V, mu, W, b, trace=False)
    return out



# revision 4
# speedup vs baseline: 1.1302x; 1.1302x over previous
"""AutoRec forward pass on 8 Trainium2 NeuronCores (SPMD, no collectives).

Computation (reference):
    z = segment_sum(r[:,None] * V[u], i, num_segments=m)   # (m, D) sparse spmm
    h = sigmoid(z + mu)
    out = sum(h[j] * W[v], -1) + b[v]                      # (n_out,)

v3 strategy (dma_gather-based; indirect_dma_start only honors one offset per
partition per instruction on real HW, so the batched row gathers go through
the SWDGE dma_gather ucode instead — int16 indices relative to a 32768-row
item chunk, thousands of rows per Pool instruction):
  - Users range-sharded over the 8 cores; user tiles of 128; tiles processed
    in groups of 8 so one dma_gather per (group, chunk) fetches all the
    group's V rows for that chunk (edges sorted by (tile, chunk, u), each
    (tile, chunk) run padded to a fixed per-chunk length Lc).
  - zt for the group's 8 tiles accumulate in two [128,512] PSUM tiles (4
    regions each) across the 7 chunk passes; S[e,s]=(rel==s)*rat built per
    128-edge block on DVE from a bf16 iota; sigmoid -> hT (transposed via PE)
    resident in SBUF as hresT [d, user].
  - Phase 2: pairs sharded by user window, sorted by (window, chunk, v).
    One TRANSPOSED dma_gather per (group, chunk) yields wgT [d, pair] slabs
    directly; Q[k,s] = wgT_blk^T @ hresT_window on PE; the per-pair pick
    out_k = Q[k, jrel_k] + b[v_k] is one fused tensor_tensor_reduce with the
    one-hot ET[k,s]=(iota==jrelT) and the bias as reduce init.
  - Host does the bucketing/padding/permutations and ships V/W as bf16.
"""

import math
import sys

sys.path.insert(0, "/opt/trn_rl_repo")

import numpy as np
import ml_dtypes

D = 128
M_CORES = 8
CHUNK = 32768
G = 8  # tiles per gather group

_PROGRAM_CACHE: dict = {}


def _build_program(NI, T1, Lc1, Lc2):
    """Build + compile the SPMD Bass program (identical on all cores).

    NI: item count; T1: user tiles/core; Lc1/Lc2: per-chunk padded run
    lengths (edges / pairs), tuples of multiples of 128.
    """
    import concourse.bacc as bacc
    import concourse.bass as bass
    import concourse.mybir as mybir
    from concourse.tile import TileContext

    f32 = mybir.dt.float32
    bf16 = mybir.dt.bfloat16
    i16 = mybir.dt.int16
    ALU = mybir.AluOpType
    ACT = mybir.ActivationFunctionType

    C = len(Lc1)
    NB1c = [l // 128 for l in Lc1]
    NB2c = [l // 128 for l in Lc2]
    NB1, NB2 = sum(NB1c), sum(NB2c)
    BOFF1 = np.cumsum([0] + NB1c).tolist()
    BOFF2 = np.cumsum([0] + NB2c).tolist()
    groups = [min(G, T1 - g * G) for g in range((T1 + G - 1) // G)]

    nc = bacc.Bacc("TRN2", target_bir_lowering=False, debug=False)

    V_d = nc.dram_tensor("V", [NI, D], bf16, kind="ExternalInput")
    W_d = nc.dram_tensor("W", [NI, D], bf16, kind="ExternalInput")
    mu_d = nc.dram_tensor("mu", [1, D], f32, kind="ExternalInput")
    eidx_d = [nc.dram_tensor(f"eidx{c}", [len(groups), 128, G * Lc1[c] // 16],
                             i16, kind="ExternalInput") for c in range(C)]
    erel_d = nc.dram_tensor("erel", [T1, 128, NB1], f32, kind="ExternalInput")
    erat_d = nc.dram_tensor("erat", [T1, 128, NB1], f32, kind="ExternalInput")
    pv_d = [nc.dram_tensor(f"pv{c}", [len(groups), 128, G * Lc2[c] // 16],
                           i16, kind="ExternalInput") for c in range(C)]
    pjrelT_d = nc.dram_tensor("pjrelT", [T1, 128, NB2], f32, kind="ExternalInput")
    pbT_d = nc.dram_tensor("pbT", [T1, 128, NB2], f32, kind="ExternalInput")
    res_d = nc.dram_tensor("res", [T1, 128, NB2], f32, kind="ExternalOutput")

    with TileContext(nc) as tc:
        with tc.tile_pool(name="const", bufs=1) as constp:
            iota_i = constp.tile([128, 128], mybir.dt.int32)
            nc.gpsimd.iota(iota_i[:], pattern=[[1, 128]], base=0, channel_multiplier=0)
            iota_bf = constp.tile([128, 128], bf16)
            nc.vector.tensor_copy(iota_bf[:], iota_i[:])
            iotac_i = constp.tile([128, 1], mybir.dt.int32)
            nc.gpsimd.iota(iotac_i[:], pattern=[[1, 1]], base=0, channel_multiplier=1)
            iotac_f = constp.tile([128, 1], f32)
            nc.vector.tensor_copy(iotac_f[:], iotac_i[:])
            ident_bf = constp.tile([128, 128], bf16)
            nc.vector.tensor_scalar(out=ident_bf[:], in0=iota_bf[:],
                                    scalar1=iotac_f[:], scalar2=None,
                                    op0=ALU.is_equal)
            ones1 = constp.tile([1, 128], f32)
            nc.vector.memset(ones1[:], 1.0)
            mu_t = constp.tile([1, 128], f32)
            nc.sync.dma_start(out=mu_t[:], in_=mu_d[:])
            hresT = constp.tile([128, T1 * 128], bf16)  # h transposed: [d, user]

            # ---- phase 1
            with tc.tile_pool(name="p1meta", bufs=2 * G + 2) as mp, \
                 tc.tile_pool(name="p1ix", bufs=3) as ixp, \
                 tc.tile_pool(name="p1g", bufs=2) as gp, \
                 tc.tile_pool(name="p1s", bufs=8) as spool, \
                 tc.tile_pool(name="p1h", bufs=4) as hp, \
                 tc.tile_pool(name="p1z", bufs=2, space="PSUM") as pp, \
                 tc.tile_pool(name="p1t", bufs=2, space="PSUM") as ptp:
                for g, GT in enumerate(groups):
                    rels, rats = [], []
                    for tl in range(GT):
                        t = g * G + tl
                        rel = mp.tile([128, NB1], f32, tag=f"rel{tl}", name=f"rel{tl}")
                        nc.sync.dma_start(out=rel[:], in_=erel_d[t])
                        rat = mp.tile([128, NB1], f32, tag=f"rat{tl}", name=f"rat{tl}")
                        nc.sync.dma_start(out=rat[:], in_=erat_d[t])
                        rels.append(rel)
                        rats.append(rat)
                    zpa = pp.tile([128, 512], f32, tag="zpa", name="zpa")
                    zpb = pp.tile([128, 512], f32, tag="zpb", name="zpb")
                    zslice = lambda tl: (zpa if tl < 4 else zpb)[:, (tl % 4) * 128:(tl % 4 + 1) * 128]
                    for tl in range(GT):
                        nc.tensor.matmul(zslice(tl), lhsT=ones1[:], rhs=mu_t[:],
                                         start=(tl % 4 == 0), stop=False)
                    for c in range(C):
                        hi = min((c + 1) * CHUNK, NI)
                        ix = ixp.tile([128, G * Lc1[c] // 16], i16,
                                      tag="ix", name=f"ix{c}")
                        nc.sync.dma_start(out=ix[:, :GT * Lc1[c] // 16],
                                          in_=eidx_d[c][g][:, :GT * Lc1[c] // 16])
                        slab = gp.tile([128, G * NB1c[c], D], bf16,
                                       tag="sl", name=f"sl{c}")
                        nc.gpsimd.dma_gather(
                            out_ap=slab[:, :GT * NB1c[c], :],
                            in_ap=V_d[c * CHUNK:hi],
                            idxs_ap=ix[:, :GT * Lc1[c] // 16],
                            num_idxs=GT * Lc1[c], num_idxs_reg=GT * Lc1[c],
                            elem_size=D, single_packet=False)
                        for tl in range(GT):
                            for bb in range(NB1c[c]):
                                col = BOFF1[c] + bb
                                S = spool.tile([128, 128], bf16, tag="S", name="S")
                                nc.vector.tensor_scalar(
                                    out=S[:], in0=iota_bf[:],
                                    scalar1=rels[tl][:, col:col + 1],
                                    scalar2=rats[tl][:, col:col + 1],
                                    op0=ALU.is_equal, op1=ALU.mult)
                                last_a = tl == min(GT, 4) - 1
                                last_b = tl == GT - 1 and GT > 4
                                nc.tensor.matmul(
                                    zslice(tl), lhsT=S[:],
                                    rhs=slab[:, tl * NB1c[c] + bb, :],
                                    start=False,
                                    stop=(c == C - 1 and bb == NB1c[c] - 1
                                          and (last_a or last_b)))
                    for tl in range(GT):
                        t = g * G + tl
                        ht = hp.tile([128, 128], bf16, tag="ht", name="ht")
                        nc.scalar.activation(ht[:], zslice(tl), ACT.Sigmoid)
                        tp = ptp.tile([128, 128], bf16, tag="tp", name="tp")
                        nc.tensor.transpose(tp[:], ht[:], ident_bf[:])
                        nc.scalar.activation(hresT[:, t * 128:(t + 1) * 128],
                                             tp[:], ACT.Identity)

            # ---- phase 2
            with tc.tile_pool(name="p2meta", bufs=2 * G + 2) as mp2, \
                 tc.tile_pool(name="p2ix", bufs=3) as ixp2, \
                 tc.tile_pool(name="p2g", bufs=2) as gp2, \
                 tc.tile_pool(name="p2e", bufs=8) as ep, \
                 tc.tile_pool(name="p2sc", bufs=8) as scp, \
                 tc.tile_pool(name="p2r", bufs=2 * G + 2) as rp, \
                 tc.tile_pool(name="p2q", bufs=4, space="PSUM") as pq:
                for g, GT in enumerate(groups):
                    jrs, pbs, rts = [], [], []
                    for wl in range(GT):
                        t = g * G + wl
                        jr = mp2.tile([128, NB2], f32, tag=f"jr{wl}", name=f"jr{wl}")
                        nc.sync.dma_start(out=jr[:], in_=pjrelT_d[t])
                        pb = mp2.tile([128, NB2], f32, tag=f"pb{wl}", name=f"pb{wl}")
                        nc.sync.dma_start(out=pb[:], in_=pbT_d[t])
                        rt = rp.tile([128, NB2], f32, tag=f"rt{wl}", name=f"rt{wl}")
                        jrs.append(jr)
                        pbs.append(pb)
                        rts.append(rt)
                    for c in range(C):
                        hi = min((c + 1) * CHUNK, NI)
                        ix = ixp2.tile([128, G * Lc2[c] // 16], i16,
                                       tag="jx", name=f"jx{c}")
                        nc.sync.dma_start(out=ix[:, :GT * Lc2[c] // 16],
                                          in_=pv_d[c][g][:, :GT * Lc2[c] // 16])
                        wslab = gp2.tile([128, 1, G * Lc2[c]], bf16,
                                         tag="wsl", name=f"wsl{c}")
                        nc.gpsimd.dma_gather(
                            out_ap=wslab[:, :, :GT * Lc2[c]],
                            in_ap=W_d[c * CHUNK:hi],
                            idxs_ap=ix[:, :GT * Lc2[c] // 16],
                            num_idxs=GT * Lc2[c], num_idxs_reg=GT * Lc2[c],
                            elem_size=D, transpose=True, single_packet=False)
                        for wl in range(GT):
                            t = g * G + wl
                            for b0 in range(0, NB2c[c], 4):
                                nb = min(4, NB2c[c] - b0)
                                qp = pq.tile([128, 512], f32, tag="qp", name="qp")
                                for k in range(nb):
                                    bb = b0 + k
                                    nc.tensor.matmul(
                                        qp[:, k * 128:(k + 1) * 128],
                                        lhsT=wslab[:, 0, (wl * Lc2[c] + bb * 128):(wl * Lc2[c] + bb * 128 + 128)],
                                        rhs=hresT[:, t * 128:(t + 1) * 128],
                                        start=(k == 0), stop=(k == nb - 1))
                                qsb = scp.tile([128, 512], bf16, tag="qsb", name="qsb")
                                nc.scalar.activation(qsb[:, :nb * 128], qp[:, :nb * 128],
                                                     ACT.Identity)
                                for k in range(nb):
                                    col = BOFF2[c] + b0 + k
                                    # rt[:,col] = sum_s (iota==jrel)*Q[:,s] = Q[k, jrel_k]
                                    sc = scp.tile([128, 128], bf16, tag="sc", name="sc")
                                    nc.vector.scalar_tensor_tensor(
                                        out=sc[:], in0=iota_bf[:],
                                        scalar=jrs[wl][:, col:col + 1],
                                        in1=qsb[:, k * 128:(k + 1) * 128],
                                        op0=ALU.is_equal, op1=ALU.mult,
                                        accum_out=rts[wl][:, col:col + 1])
                    for wl in range(GT):
                        t = g * G + wl
                        nc.vector.tensor_add(rts[wl][:], rts[wl][:], pbs[wl][:])
                        nc.sync.dma_start(out=res_d[t], in_=rts[wl][:])

    nc.compile()
    return nc


def _pad_runs(keys, nkeys, sortvals, C, chunk_of, payloads):
    """Sort records by (key, chunk, sortval); compute per-chunk padded run
    lengths Lc and emit padded streams.

    Returns (Lc list, per-(key,chunk) slot mapping info).
    """
    order = np.lexsort((sortvals, chunk_of, keys))
    ks, cs = keys[order], chunk_of[order]
    cnt = np.zeros((nkeys, C), np.int64)
    np.add.at(cnt, (ks, cs), 1)
    Lc = [max(128, 128 * int(math.ceil(cnt[:, c].max() / 128.0))) for c in range(C)]
    return order, cnt, Lc


def _prep_inputs(u, i, r, m, v, j, V, mu, W, b):
    NU = int(m)
    NI = int(V.shape[0])
    NOUT = int(v.shape[0])
    UC = (NU + M_CORES - 1) // M_CORES
    T1 = (UC + 127) // 128
    C = (NI + CHUNK - 1) // CHUNK
    NT = M_CORES * T1

    u64 = np.asarray(u).astype(np.int64)
    i64 = np.asarray(i).astype(np.int64)
    r32 = np.asarray(r, dtype=np.float32)

    # ---- phase 1 edges: key = (core, tile); sorted by (key, chunk, u)
    c_e = i64 // UC
    tloc = (i64 - c_e * UC) >> 7
    gtile = (c_e * T1 + tloc).astype(np.int64)
    irel = (i64 - c_e * UC - (tloc << 7)).astype(np.float32)
    echunk = (u64 // CHUNK).astype(np.int64)
    order1, cnt1, Lc1 = _pad_runs(gtile, NT, u64, C, echunk, None)
    NB1c = [l // 128 for l in Lc1]
    NB1 = sum(NB1c)
    BOFF1 = np.cumsum([0] + NB1c)

    gt_s = gtile[order1]
    ch_s = echunk[order1]
    us = (u64 - echunk * CHUNK)[order1].astype(np.int16)
    rs = r32[order1]
    irel_s = irel[order1]

    # slot within the (tile, chunk) run
    runoff = np.arange(len(order1), dtype=np.int64)
    runkey = gt_s * C + ch_s
    runstart = np.zeros(len(order1), np.int64)
    newrun = np.ones(len(order1), bool)
    newrun[1:] = runkey[1:] != runkey[:-1]
    runstart[newrun] = runoff[newrun]
    runstart = np.maximum.accumulate(runstart)
    off_in_run = runoff - runstart

    NGRP = (T1 + G - 1) // G
    groups = [min(G, T1 - g * G) for g in range(NGRP)]

    # idx streams per (core, grp, chunk): [128, G*Lc/16] int16 (wrap-16, x8)
    eidx = [np.zeros((M_CORES, NGRP, 128, G * Lc1[c] // 16), np.int16)
            for c in range(C)]
    erel = np.zeros((M_CORES, T1, 128, NB1), np.float32)
    erat = np.zeros((M_CORES, T1, 128, NB1), np.float32)

    core_s = gt_s // T1
    tile_s = gt_s % T1
    tl_s = tile_s % G
    grp_s = tile_s // G
    # position of each edge within its slab idx stream
    pos_in_slab = tl_s * np.array(Lc1)[ch_s] + off_in_run
    for c in range(C):
        mask = ch_s == c
        n16 = pos_in_slab[mask] // 16
        p16 = pos_in_slab[mask] % 16
        for grp_rep in range(8):
            eidx[c][core_s[mask], grp_s[mask], grp_rep * 16 + p16, n16] = us[mask]
    blk = BOFF1[ch_s] + off_in_run // 128
    prt = off_in_run % 128
    erel[core_s, tile_s, prt, blk] = irel_s
    erat[core_s, tile_s, prt, blk] = rs

    # ---- phase 2 pairs
    j64 = np.asarray(j).astype(np.int64)
    v64 = np.asarray(v).astype(np.int64)
    bvec = np.asarray(b, dtype=np.float32).reshape(-1)
    cj = j64 // UC
    tj = (j64 - cj * UC) >> 7
    gwin = (cj * T1 + tj).astype(np.int64)
    jrel = (j64 - cj * UC - (tj << 7)).astype(np.float32)
    pchunk = (v64 // CHUNK).astype(np.int64)
    order2, cnt2, Lc2 = _pad_runs(gwin, NT, v64, C, pchunk, None)
    NB2c = [l // 128 for l in Lc2]
    NB2 = sum(NB2c)
    BOFF2 = np.cumsum([0] + NB2c)

    gw_s = gwin[order2]
    ch2_s = pchunk[order2]
    vs = (v64 - pchunk * CHUNK)[order2].astype(np.int16)
    jr_s = jrel[order2]
    pb_s = bvec[v64[order2]]

    runoff = np.arange(len(order2), dtype=np.int64)
    runkey = gw_s * C + ch2_s
    runstart = np.zeros(len(order2), np.int64)
    newrun = np.ones(len(order2), bool)
    newrun[1:] = runkey[1:] != runkey[:-1]
    runstart[newrun] = runoff[newrun]
    runstart = np.maximum.accumulate(runstart)
    off2 = runoff - runstart

    pv = [np.zeros((M_CORES, NGRP, 128, G * Lc2[c] // 16), np.int16)
          for c in range(C)]
    pjrelT = np.zeros((M_CORES, T1, 128, NB2), np.float32)
    pbT = np.zeros((M_CORES, T1, 128, NB2), np.float32)

    core2 = gw_s // T1
    win2 = gw_s % T1
    wl2 = win2 % G
    grp2 = win2 // G
    pos2 = wl2 * np.array(Lc2)[ch2_s] + off2
    for c in range(C):
        mask = ch2_s == c
        n16 = pos2[mask] // 16
        p16 = pos2[mask] % 16
        for grp_rep in range(8):
            pv[c][core2[mask], grp2[mask], grp_rep * 16 + p16, n16] = vs[mask]
    blk2 = BOFF2[ch2_s] + off2 // 128
    prt2 = off2 % 128
    pjrelT[core2, win2, prt2, blk2] = jr_s
    pbT[core2, win2, prt2, blk2] = pb_s
    # slot of each sorted pair in res [T1,128,NB2] flat (per core)
    slot2 = win2 * (128 * NB2) + prt2 * NB2 + blk2

    Vb = np.asarray(V, dtype=np.float32).astype(ml_dtypes.bfloat16)
    Wb = np.asarray(W, dtype=np.float32).astype(ml_dtypes.bfloat16)
    muf = np.ascontiguousarray(np.asarray(mu).reshape(1, D), dtype=np.float32)

    in_maps = []
    for c0 in range(M_CORES):
        im = {"V": Vb, "W": Wb, "mu": muf,
              "erel": erel[c0], "erat": erat[c0],
              "pjrelT": pjrelT[c0], "pbT": pbT[c0]}
        for c in range(C):
            im[f"eidx{c}"] = eidx[c][c0]
            im[f"pv{c}"] = pv[c][c0]
        in_maps.append(im)
    meta = dict(NI=NI, T1=T1, Lc1=tuple(Lc1), Lc2=tuple(Lc2), NB2=NB2,
                NOUT=NOUT, order2=order2, core2=core2, slot2=slot2)
    return in_maps, meta


def _unshard(results, meta):
    out = np.empty(meta["NOUT"], np.float32)
    vals = np.empty(len(meta["order2"]), np.float32)
    for c0 in range(M_CORES):
        mask = meta["core2"] == c0
        flat = results[c0]["res"].reshape(-1)
        vals[mask] = flat[meta["slot2"][mask]]
    out[meta["order2"]] = vals
    return out


def run(u, i, r, m, v, j, V, mu, W, b, trace=False, trace_kwargs=None):
    from concourse import bass_utils

    in_maps, meta = _prep_inputs(u, i, r, m, v, j, V, mu, W, b)
    key = (meta["NI"], meta["T1"], meta["Lc1"], meta["Lc2"])
    nc = _PROGRAM_CACHE.get(key)
    if nc is None:
        nc = _build_program(*key)
        _PROGRAM_CACHE[key] = nc
    res = bass_utils.run_bass_kernel_spmd(
        nc, in_maps, list(range(M_CORES)), trace=trace, **(trace_kwargs or {}))
    return _unshard(res.results, meta), res


def kernel(u, i, r, m, v, j, V, mu, W, b):
    out, _ = run(u, i, r, m, v, j, V, mu, W, b, trace=False)
    return out
